# revision 54
# baseline (speedup 1.0000x reference)
"""Trainium2 Bass kernel for nn_Attention_35923106463893.

Multi-head attention block:
    qkv = (weight[:, :, None] * (x @ W_qkv)) -> split q,k,v over 12 heads
    A = softmax(q k^T / sqrt(64));  out = (A v) reshaped @ W_msa + b_msa

Sharding: pure data-parallel over batch B=8 -> one batch element per
NeuronCore, no collectives.

Structure (all matmul operands bf16; PSUM accumulation fp32):
- qk^T computed in transposed layout (head-dims on partitions) as 12
  [128, N] chunks; token gate folded into the PSUM->SBUF copies.
- Per-head scores S^T [keys, queries] -> ACT exp -> E bf16.
- A@V runs re-oriented: out[q, 65] accumulates E^T-stationary x [V|1]
  moving (65-row matmuls are charged by moving size only), so the 12
  heads cost ~21us instead of ~41us on PE. The ones column of V makes
  psum col 64 the softmax denominator; normalization is a per-partition
  reciprocal + scalar-mul.
- Normalized attention transposes back to [d, tokens] for the output
  projection via DMA-XBAR transpose instructions (off the compute
  engines). The final head pair transposes via PE at the tail.
- Output projection y^T = W_msa^T attn^T accumulates per (c, j) in
  PSUM; k=0..4 partials run during heads 10-11 and are copied (bias
  folded in) to SBUF, so after head 11 only an identity re-inject
  + k=5 step + copy + bf16 DMA remain.

Scheduling: the main loop emits S/exp per (head, r-chunk) slot; all
other PE work (V projection, O accumulation steps, qk chunks,
projection partials, normalizations) lives in one ordered filler chain
whose generators yield their PE cost in ns. Each slot consumes filler
work against a per-head budget, keeping the in-order PE stream dense
without head-of-line blocking.
"""

from contextlib import ExitStack
from itertools import zip_longest

import numpy as np
import ml_dtypes

import concourse.bass as bass
import concourse.mybir as mybir
import concourse.tile as tile
from concourse import bacc
from concourse.bass import ts
from concourse.bass_utils import run_bass_kernel_spmd

B, N, D, H = 8, 1024, 768, 12
HD = D // H          # 64
HE = HD + 1          # 65: head value dims + ones column
SCALE = HD ** -0.5   # 0.125
KC = D // 128        # 6 contraction chunks
NT = N // 128        # 8 token chunks
NC2 = N // 512       # 2 moving chunks

F32 = mybir.dt.float32
F32R = mybir.dt.float32r
BF16 = mybir.dt.bfloat16
AF = mybir.ActivationFunctionType

_CACHE: dict = {}
BLOCK = 10**9  # filler-chain sentinel: "waiting on the main loop"
# per-head filler budgets (ns of planned PE work per r-slot), tuned on the
# timeline model
BUDGETS = [700, 1250, 1250, 700, 650, 650, 650, 650, 650, 800, 1300, 1300]
BRIDGE = 6  # junk matmuls bridging the first-S gate-copy latency


def _run(gen):
    for _ in gen:
        pass


def _emit(tc, repeat=1):
    nc = tc.nc
    xt_d = nc.dram_tensor("xt", [D, N], BF16, kind="ExternalInput").ap()
    w_d = nc.dram_tensor("w", [1, N], F32, kind="ExternalInput").ap()
    # chunk-major packed [q;k]^T weights: row block m holds the six
    # [128, 128] stationary pieces of qk chunk m side by side
    wqk_d = nc.dram_tensor("wqk", [2 * D, D], BF16, kind="ExternalInput").ap()
    wv_d = nc.dram_tensor("wv", [D, D], BF16, kind="ExternalInput").ap()
    wm_d = nc.dram_tensor("wmsa", [D, D], BF16, kind="ExternalInput").ap()
    bm_d = nc.dram_tensor("bmsa", [D], F32, kind="ExternalInput").ap()
    id_d = nc.dram_tensor("ident", [128, 128], BF16, kind="ExternalInput").ap()
    y_d = nc.dram_tensor("yt", [D, N], BF16, kind="ExternalOutput").ap()

    for _rep in range(repeat):
        _emit_body(tc, xt_d, w_d, wqk_d, wv_d, wm_d, bm_d, id_d, y_d)


def _emit_body(tc, xt_d, w_d, wqk_d, wv_d, wm_d, bm_d, id_d, y_d):
    nc = tc.nc
    with ExitStack() as s1:
        const = s1.enter_context(tc.tile_pool(name="const", bufs=1))
        pwm = s1.enter_context(tc.tile_pool(name="pwm", bufs=1))
        pwv = s1.enter_context(tc.tile_pool(name="pwv", bufs=1))
        pqk = s1.enter_context(tc.tile_pool(name="pqk", bufs=1))
        pv = s1.enter_context(tc.tile_pool(name="pv", bufs=1))
        pot = s1.enter_context(tc.tile_pool(name="pot", bufs=1))
        pxt = s1.enter_context(tc.tile_pool(name="pxt", bufs=1))
        pwqs = s1.enter_context(tc.tile_pool(name="pwqs", bufs=1))
        pe_ = s1.enter_context(tc.tile_pool(name="pe", bufs=12))
        pstg = s1.enter_context(tc.tile_pool(name="pstg", bufs=2))
        prc = s1.enter_context(tc.tile_pool(name="prc", bufs=6))
        pp1 = s1.enter_context(tc.tile_pool(name="pp1", bufs=1))
        pdn = s1.enter_context(tc.tile_pool(name="pdn", bufs=1))
        pfin = s1.enter_context(tc.tile_pool(name="pfin", bufs=12))
        psA = s1.enter_context(tc.tile_pool(name="psA", bufs=2, space="PSUM"))
        psO = s1.enter_context(tc.tile_pool(name="psO", bufs=2, space="PSUM"))
        psP = s1.enter_context(tc.tile_pool(name="psP", bufs=2, space="PSUM"))

        # ---- startup loads ----
        # HWDGE charges ~632ns fixed per DMA, so loads are batched: full
        # x^T tiles, and one strided gather each for W_v / the remaining
        # qk chunks / W_msa. Queue order controls landing order.
        wq0 = pwqs.tile([128, KC * 128], BF16, tag="wq0", name="wq0")
        wq6 = pwqs.tile([128, KC * 128], BF16, tag="wq6", name="wq6")
        wqr = [
            pwqs.tile([128, 5 * KC * 128], BF16, tag=f"wqr{i}", name=f"wqr{i}")
            for i in range(2)
        ]
        w_row = pdn.tile([1, N], F32, tag="dn", name="w_row")
        nc.scalar.dma_start(w_row[:], w_d[:])
        wcol = const.tile([128, NT], F32, tag="wcol")
        nc.sync.dma_start(wcol[:], w_d[0, :].rearrange("(r p) -> p r", p=128))
        nc.scalar.dma_start(wq0[:], wqk_d[0:128, :])
        xtt = [pxt.tile([128, N], BF16, tag=f"xt{c}", name=f"xt{c}") for c in range(KC)]
        nc.sync.dma_start(xtt[0][:], xt_d[0:128, :])
        nc.scalar.dma_start(wq6[:], wqk_d[ts(KC, 128), :])
        for c in range(1, KC):
            eng = nc.sync if c % 2 == 0 else nc.scalar
            eng.dma_start(xtt[c][:], xt_d[ts(c, 128), :])
        wqk3 = wqk_d.rearrange("(m p) e -> p m e", p=128)
        wvb = pwv.tile([128, KC * D], BF16, tag="wvb", name="wvb")
        nc.sync.dma_start(
            wvb[:].rearrange("p (c e) -> p c e", e=D),
            wv_d.rearrange("(c p) e -> p c e", p=128),
        )
        nc.scalar.dma_start(
            wqr[0][:].rearrange("p (m e) -> p m e", e=D), wqk3[:, 1:KC, :]
        )
        nc.sync.dma_start(
            wqr[1][:].rearrange("p (m e) -> p m e", e=D),
            wqk3[:, KC + 1 : 2 * KC, :],
        )
        bias = const.tile([128, KC], F32, tag="bias")
        nc.scalar.dma_start(bias[:], bm_d[:].rearrange("(c p) -> p c", p=128))
        ident = const.tile([128, 128], BF16, tag="ident")
        nc.scalar.dma_start(ident[:], id_d[:])

        def wqt(m):
            if m == 0:
                return wq0[:]
            if m == KC:
                return wq6[:]
            i, off = (0, m - 1) if m < KC else (1, m - KC - 1)
            return wqr[i][:, off * D : (off + 1) * D]

        def wvt(c):
            return wvb[:, c * D : (c + 1) * D]

        ones_bf = const.tile([128, HD], BF16, tag="ones_bf")
        nc.vector.memset(ones_bf[:], 1.0)
        jconst = const.tile([128, 512], BF16, tag="jconst")
        nc.vector.memset(jconst[:], 0.5)

        # ---- PE warm-up: the clock gate holds PE at reduced speed until
        # ~3us of sustained activity; junk matmuls on resident const tiles
        # cover the initial DMA wait so real work starts at full clock.
        psj = psA.tile([128, N], F32, tag="psA", name="psj")
        for _ in range(2):
            nc.tensor.matmul(
                psj[0:HD, 0:HD], ones_bf[:], ones_bf[:], start=True, stop=True
            )
        for _ in range(9):
            nc.tensor.matmul(
                psj[0:HD, 0:512], ones_bf[:], jconst[:], start=True, stop=True
            )

        wb = const.tile([128, N], F32, tag="wb")
        nc.gpsimd.partition_broadcast(wb[:], w_row[:])
        # per-key token gate folded into the exp scale (per-partition AP)
        wcolS = const.tile([128, NT], F32, tag="wcolS")
        nc.vector.tensor_scalar_mul(wcolS[:], wcol[:], SCALE)

        qkt = [pqk.tile([128, N], BF16, tag=f"qk{m}", name=f"qk{m}") for m in range(2 * KC)]
        vt = [pv.tile([128, H * HE], BF16, tag=f"v{r}", name=f"v{r}") for r in range(NT)]
        ott = [pot.tile([128, N], BF16, tag=f"ot{c}", name=f"ot{c}") for c in range(KC)]
        wmb = pwm.tile([128, KC * D], BF16, tag="wmb", name="wmb")

        def wmt(k):
            return wmb[:, k * D : (k + 1) * D]
        p1t = [
            pp1.tile([128, 512], BF16, tag=f"p1_{g}", name=f"p1_{g}")
            for g in range(2 * KC)
        ]

        # shared state between the main S/exp loop and the filler gens
        e_store: dict = {}   # (h, r) -> e tile
        o_ps: dict = {}      # h -> (oa, ob)
        stg_cur: dict = {}   # qc -> pair staging tile
        stg5: dict = {}      # qc -> pair-5 staging tile

        def gen_qk(m, pool=None):
            """qk^T chunk m. Yields (per contraction step) its PE cost.

            q chunks (m < KC) fold the token gate in via a DVE multiply;
            k chunks copy plainly on ACT -- their gate rides in the exp
            scale argument (per-partition = per-key)."""
            wq_m = wqt(m)
            pool = pool or psP
            tag = "psP" if pool is psP else "psO"
            ps0 = pool.tile([128, 512], F32, tag=tag, name="psP")
            ps1 = pool.tile([128, 512], F32, tag=tag, name="psP")
            pj = (ps0, ps1)
            for c in range(KC):
                for j in range(NC2):
                    nc.tensor.matmul(
                        pj[j][:],
                        wq_m[:, ts(c, 128)],
                        xtt[c][:, ts(j, 512)],
                        start=(c == 0),
                        stop=(c == KC - 1),
                    )
                yield 427
            yield 0
            for j in range(NC2):
                nc.vector.tensor_mul(
                    qkt[m][:, ts(j, 512)], pj[j][:], wb[:, ts(j, 512)]
                )

        def gen_v():
            """V projection, fine-grained: yields per (r, c) step."""
            for r in range(NT):
                ta = psP.tile([128, 512], F32, tag="psP", name="psP")
                tb = psP.tile([128, 512], F32, tag="psP", name="psP")
                for c in range(KC):
                    nc.tensor.matmul(
                        ta[:],
                        xtt[c][:, ts(r, 128)],
                        wvt(c)[:, 0:512],
                        start=(c == 0),
                        stop=(c == KC - 1),
                    )
                    nc.tensor.matmul(
                        tb[:, 0:256],
                        xtt[c][:, ts(r, 128)],
                        wvt(c)[:, 512:768],
                        start=(c == 0),
                        stop=(c == KC - 1),
                    )
                    yield 320
                v3 = vt[r][:].rearrange("p (h e) -> p h e", e=HE)
                nc.vector.tensor_copy(
                    v3[:, :, HD : HD + 1],
                    ones_bf[:, 0:H].rearrange("p (h o) -> p h o", o=1),
                )
                nc.vector.tensor_scalar_mul(
                    v3[:, 0:8, 0:HD],
                    ta[:].rearrange("p (h e) -> p h e", e=HD),
                    wcol[:, r : r + 1],
                )
                nc.vector.tensor_scalar_mul(
                    v3[:, 8:12, 0:HD],
                    tb[:, 0:256].rearrange("p (h e) -> p h e", e=HD),
                    wcol[:, r : r + 1],
                )
                yield 50

        def gen_o(h):
            """A@V accumulation steps for head h; one yield per r-chunk."""
            oa = psO.tile([128, 512], F32, tag="psO", name="psO")
            ob = psO.tile([128, 512], F32, tag="psO", name="psO")
            o_ps[h] = (oa, ob)
            for r in range(NT):
                while (h, r) not in e_store:
                    yield BLOCK
                e = e_store.pop((h, r))
                for qc in range(NT):
                    po = oa if qc < 4 else ob
                    off = HE * (qc % 4)
                    nc.tensor.matmul(
                        po[:, off : off + HE],
                        e[:, ts(qc, 128)],
                        vt[r][:, h * HE : (h + 1) * HE],
                        start=(r == 0 and qc % 4 == 0),
                        stop=(r == NT - 1),
                    )
                yield 217

        def gen_norm(h):
            """normalize head h into pair staging; odd heads also emit the
            pair's XBAR transposes into ott. One yield per token chunk."""
            oa, ob = o_ps.pop(h)
            for qc in range(NT):
                po = oa if qc < 4 else ob
                off = HE * (qc % 4)
                rc = prc.tile([128, 1], F32, tag="rc", name="rc")
                nc.vector.reciprocal_approx_fast(
                    rc[:], po[:, off + HD : off + HD + 1]
                )
                stg = stg_cur
                if h >= 10:
                    stg = stg5
                if h % 2 == 0:
                    stg[qc] = pstg.tile(
                        [128, 128], BF16, tag=f"stg{qc}", name=f"stg{qc}"
                    )
                nc.vector.tensor_scalar_mul(
                    stg[qc][:, HD * (h % 2) : HD * (h % 2) + HD],
                    po[:, off : off + HD],
                    rc[:, 0:1],
                )
                if h % 2 == 1 and h < 10:
                    nc.sync.dma_start_transpose(
                        ott[h // 2][:, ts(qc, 128)], stg[qc][:]
                    )
                yield 60

        def gen_stg5_transposes():
            """head-10 halves of pair 5 ride to ott[5] rows 0:64 (cols 64:128
            carry stale data; the tail rewrites rows 64:128)."""
            for qc in range(NT):
                nc.sync.dma_start_transpose(ott[KC - 1][:, ts(qc, 128)], stg5[qc][:])
                yield 60

        def gen_p1(c, j):
            """output-projection partial k=0..4 for group (c, j), in two
            lumps; the copy folds the bias in and frees the bank."""
            ps = psP.tile([128, 512], F32, tag="psP", name="psP")
            for k in range(3):
                nc.tensor.matmul(
                    ps[:],
                    wmt(k)[:, ts(c, 128)],
                    ott[k][:, ts(j, 512)],
                    start=(k == 0),
                    stop=False,
                )
            yield 640
            for k in range(3, KC - 1):
                nc.tensor.matmul(
                    ps[:],
                    wmt(k)[:, ts(c, 128)],
                    ott[k][:, ts(j, 512)],
                    start=False,
                    stop=(k == KC - 2),
                )
            nc.vector.tensor_scalar_add(p1t[2 * c + j][:], ps[:], bias[:, c : c + 1])
            yield 427

        def gen_wmt_loads():
            nc.sync.dma_start(
                wmb[:].rearrange("p (c e) -> p c e", e=D),
                wm_d.rearrange("(c p) e -> p c e", p=128),
            )
            yield 0

        # ---- ordered filler chain ----
        def filler_chain():
            yield from gen_qk(1)
            yield from gen_qk(KC + 1)
            yield from gen_v()
            yield from gen_wmt_loads()
            yield from gen_o(0)
            yield from gen_norm(0)
            yield from gen_qk(2)
            yield from gen_o(1)
            yield from gen_norm(1)
            yield from gen_qk(KC + 2)
            yield from gen_o(2)
            yield from gen_norm(2)
            yield from gen_qk(3)
            yield from gen_o(3)
            yield from gen_norm(3)
            yield from gen_qk(KC + 3)
            yield from gen_o(4)
            yield from gen_norm(4)
            yield from gen_qk(4)
            yield from gen_o(5)
            yield from gen_norm(5)
            yield from gen_qk(KC + 4)
            yield from gen_o(6)
            yield from gen_norm(6)
            yield from gen_qk(5)
            yield from gen_o(7)
            yield from gen_norm(7)
            yield from gen_qk(KC + 5)
            yield from gen_o(8)
            yield from gen_norm(8)
            yield from gen_o(9)
            yield from gen_norm(9)
            for c in range(KC):
                yield from gen_p1(c, 0)
            yield from gen_o(10)
            yield from gen_norm(10)
            yield from gen_stg5_transposes()
            for c in range(KC):
                yield from gen_p1(c, 1)

        fill_src = filler_chain()
        owed = [0.0]

        def fill(ns):
            owed[0] += ns
            while owed[0] > 0:
                c = next(fill_src, None)
                if c is None:
                    owed[0] = 0
                    return
                if c >= BLOCK:
                    # filler is waiting on the main loop; retry next slot
                    owed[0] = 0
                    return
                owed[0] -= max(c, 40)

        # per-head filler budget (ns of planned PE work per r-slot)
        budgets = dict(enumerate(BUDGETS))

        # ---- main S/exp loop ----
        # chunks 0 and 6 interleave c-steps so PE tracks the x^T arrivals
        for _steps in zip_longest(gen_qk(0), gen_qk(KC, pool=psO)):
            pass
        # bridge the qkt gate-copy latency with junk so the PE clock ramp
        # never resets before the first S matmul
        psj2 = psA.tile([128, N], F32, tag="psA", name="psj2")
        for _ in range(BRIDGE):
            nc.tensor.matmul(
                psj2[0:HD, 0:512], ones_bf[:], jconst[:], start=True, stop=True
            )

        for h in range(H):
            qt, qr = qkt[h // 2], HD * (h % 2)
            kt = qkt[KC + h // 2]
            for r in range(NT):
                ps = psA.tile([128, N], F32, tag="psA", name="psA")
                for j in range(NC2):
                    nc.tensor.matmul(
                        ps[:, ts(j, 512)],
                        kt[qr : qr + HD, ts(r, 128)],
                        qt[qr : qr + HD, ts(j, 512)],
                        start=True,
                        stop=True,
                    )
                e = pe_.tile([128, N], BF16, tag="e", name="e")
                nc.scalar.activation(e[:], ps[:], AF.Exp, scale=SCALE)
                e_store[(h, r)] = e
                fill(budgets[h])

        # ---- tail: head-11 O + norm + PE transposes + final projection ----
        fill(10**9)  # drain any remaining filler work
        oa = psO.tile([128, 512], F32, tag="psO", name="psO")
        ob = psO.tile([128, 512], F32, tag="psO", name="psO")
        for r in range(NT):
            e = e_store.pop((11, r))
            for qc in range(NT):
                po = oa if qc < 4 else ob
                off = HE * (qc % 4)
                nc.tensor.matmul(
                    po[:, off : off + HE],
                    e[:, ts(qc, 128)],
                    vt[r][:, 11 * HE : 12 * HE],
                    start=(r == 0 and qc % 4 == 0),
                    stop=(r == NT - 1),
                )
        # pre-inject the first two partials while the norm chain runs
        psl = [psP.tile([128, 512], F32, tag="psP", name="psF")[:] for _ in range(2)]
        for g in range(2):
            nc.tensor.matmul(psl[g], ident[:], p1t[2 * g][:], start=True, stop=False)
        trt = None
        for qc in range(NT):
            po = oa if qc < 4 else ob
            off = HE * (qc % 4)
            rc = prc.tile([128, 1], F32, tag="rc", name="rc")
            nc.vector.reciprocal_approx_fast(rc[:], po[:, off + HD : off + HD + 1])
            s11 = pstg.tile([128, HD], BF16, tag="s11", name="s11", bufs=4)
            nc.scalar.activation(
                s11[:], po[:, off : off + HD], AF.Copy, scale=rc[:, 0:1]
            )
            if qc % 2 == 0:
                trt = psA.tile([128, N], F32, tag="psA", name="psA")
            dst = trt[0:HD, 512 * (qc % 2) : 512 * (qc % 2) + HD].bitcast(BF16)
            nc.tensor.transpose(dst, s11[:], ident[:])
            nc.vector.tensor_copy(ott[KC - 1][HD:128, ts(qc, 128)], dst)
        # final projection step: re-inject partials, add k=5, copy, store.
        # Groups rotate across psP(2) + psO(2) + two psA tiles (4 banks).
        trp = [psA.tile([128, N], F32, tag="psA", name="psA") for _ in range(2)]
        slots = (
            psl
            + [psO.tile([128, 512], F32, tag="psO", name="psF")[:] for _ in range(2)]
            + [t[:, ts(j, 512)] for t in trp for j in range(2)]
        )
        g = 0
        for j in range(NC2):
            for c in range(KC):
                ps = slots[g % 8]
                if g >= 2:
                    nc.tensor.matmul(
                        ps, ident[:], p1t[2 * c + j][:], start=True, stop=False
                    )
                nc.tensor.matmul(
                    ps,
                    wmt(KC - 1)[:, ts(c, 128)],
                    ott[KC - 1][:, ts(j, 512)],
                    start=False,
                    stop=True,
                )
                fin = pfin.tile([128, 512], BF16, tag="fin", name="fin")
                if g % 2 == 0:
                    nc.scalar.activation(fin[:], ps, AF.Copy)
                else:
                    nc.vector.tensor_copy(fin[:], ps)
                eng = nc.sync if g % 2 == 0 else nc.scalar
                eng.dma_start(y_d[ts(c, 128), ts(j, 512)], fin[:])
                g += 1


def _build(repeat=1):
    key = ("nc", repeat)
    if key not in _CACHE:
        nc = bacc.Bacc("TRN2", target_bir_lowering=False, debug=False, num_devices=B)
        with tile.TileContext(nc) as tc:
            _emit(tc, repeat=repeat)
        nc.compile()
        _CACHE[key] = nc
    return _CACHE[key]


def kernel(x, weight, W_qkv, W_msa, b_msa):
    nc = _build()
    x = np.asarray(x, dtype=np.float32)
    weight = np.asarray(weight, dtype=np.float32)
    W_qkv = np.asarray(W_qkv, dtype=np.float32)
    bf = ml_dtypes.bfloat16
    # pack [q;k]^T chunk-major: row block m = the six [128,128] stationary
    # pieces of qk chunk m side by side (see _emit)
    wqk = np.ascontiguousarray(
        W_qkv[:, : 2 * D]
        .reshape(KC, 128, 2 * KC, 128)
        .transpose(2, 1, 0, 3)
        .reshape(2 * D, D)
    ).astype(bf)
    wv = np.ascontiguousarray(W_qkv[:, 2 * D :]).astype(bf)
    wm = np.asarray(W_msa, dtype=np.float32).astype(bf)
    ident = np.eye(128, dtype=np.float32).astype(bf)
    in_maps = []
    for b in range(B):
        in_maps.append(
            {
                "xt": np.ascontiguousarray(x[b].T).astype(bf),
                "w": np.ascontiguousarray(weight[b : b + 1]),
                "wqk": wqk,
                "wv": wv,
                "wmsa": wm,
                "bmsa": np.asarray(b_msa, dtype=np.float32),
                "ident": ident,
            }
        )
    res = run_bass_kernel_spmd(nc, in_maps, list(range(B)))
    out = np.stack(
        [res.results[b]["yt"].astype(np.float32).T for b in range(B)], axis=0
    )
    return np.ascontiguousarray(out)


# revision 56
# speedup vs baseline: 1.0507x; 1.0507x over previous
"""Trainium2 Bass kernel for nn_Attention_35923106463893.

Multi-head attention block:
    qkv = (weight[:, :, None] * (x @ W_qkv)) -> split q,k,v over 12 heads
    A = softmax(q k^T / sqrt(64));  out = (A v) reshaped @ W_msa + b_msa

Sharding: pure data-parallel over batch B=8 -> one batch element per
NeuronCore, no collectives.

Structure (all matmul operands bf16; PSUM accumulation fp32):
- qk^T computed in transposed layout (head-dims on partitions) as 12
  [128, N] chunks; token gate folded into the PSUM->SBUF copies.
- Per-head scores S^T [keys, queries] -> ACT exp -> E bf16.
- A@V runs re-oriented: out[q, 65] accumulates E^T-stationary x [V|1]
  moving (65-row matmuls are charged by moving size only), so the 12
  heads cost ~21us instead of ~41us on PE. The ones column of V makes
  psum col 64 the softmax denominator; normalization is a per-partition
  reciprocal + scalar-mul.
- Normalized attention transposes back to [d, tokens] for the output
  projection via DMA-XBAR transpose instructions (off the compute
  engines). The final head pair transposes via PE at the tail.
- Output projection y^T = W_msa^T attn^T accumulates per (c, j) in
  PSUM; k=0..4 partials run during heads 10-11 and are copied (bias
  folded in) to SBUF, so after head 11 only an identity re-inject
  + k=5 step + copy + bf16 DMA remain.

Scheduling: the main loop emits S/exp per (head, r-chunk) slot; all
other PE work (V projection, O accumulation steps, qk chunks,
projection partials, normalizations) lives in one ordered filler chain
whose generators yield their PE cost in ns. Each slot consumes filler
work against a per-head budget, keeping the in-order PE stream dense
without head-of-line blocking.
"""

from contextlib import ExitStack
from itertools import zip_longest

import numpy as np
import ml_dtypes

import concourse.bass as bass
import concourse.mybir as mybir
import concourse.tile as tile
from concourse import bacc
from concourse.bass import ts
from concourse.bass_utils import run_bass_kernel_spmd

B, N, D, H = 8, 1024, 768, 12
HD = D // H          # 64
HE = HD + 1          # 65: head value dims + ones column
SCALE = HD ** -0.5   # 0.125
KC = D // 128        # 6 contraction chunks
NT = N // 128        # 8 token chunks
NC2 = N // 512       # 2 moving chunks

F32 = mybir.dt.float32
F32R = mybir.dt.float32r
BF16 = mybir.dt.bfloat16
AF = mybir.ActivationFunctionType

_CACHE: dict = {}
BLOCK = 10**9  # filler-chain sentinel: "waiting on the main loop"
# per-head filler budgets (ns of planned PE work per r-slot), tuned on the
# timeline model
BUDGETS = [700, 1250, 1250, 700, 650, 650, 650, 650, 650, 800, 1300, 1300]
BRIDGE = 6  # junk matmuls bridging the first-S gate-copy latency


def _run(gen):
    for _ in gen:
        pass


def _emit(tc, repeat=1):
    nc = tc.nc
    xt_d = nc.dram_tensor("xt", [D, N], BF16, kind="ExternalInput").ap()
    w_d = nc.dram_tensor("w", [1, N], F32, kind="ExternalInput").ap()
    # chunk-major packed [q;k]^T weights: row block m holds the six
    # [128, 128] stationary pieces of qk chunk m side by side
    wqk_d = nc.dram_tensor("wqk", [2 * D, D], BF16, kind="ExternalInput").ap()
    wv_d = nc.dram_tensor("wv", [D, D], BF16, kind="ExternalInput").ap()
    wm_d = nc.dram_tensor("wmsa", [D, D], BF16, kind="ExternalInput").ap()
    bm_d = nc.dram_tensor("bmsa", [D], F32, kind="ExternalInput").ap()
    id_d = nc.dram_tensor("ident", [128, 128], BF16, kind="ExternalInput").ap()
    y_d = nc.dram_tensor("yt", [D, N], BF16, kind="ExternalOutput").ap()

    for _rep in range(repeat):
        _emit_body(tc, xt_d, w_d, wqk_d, wv_d, wm_d, bm_d, id_d, y_d)


def _emit_body(tc, xt_d, w_d, wqk_d, wv_d, wm_d, bm_d, id_d, y_d):
    nc = tc.nc
    with ExitStack() as s1:
        const = s1.enter_context(tc.tile_pool(name="const", bufs=1))
        pwm = s1.enter_context(tc.tile_pool(name="pwm", bufs=1))
        pwv = s1.enter_context(tc.tile_pool(name="pwv", bufs=1))
        pqk = s1.enter_context(tc.tile_pool(name="pqk", bufs=1))
        pv = s1.enter_context(tc.tile_pool(name="pv", bufs=1))
        pot = s1.enter_context(tc.tile_pool(name="pot", bufs=1))
        pxt = s1.enter_context(tc.tile_pool(name="pxt", bufs=1))
        pwqs = s1.enter_context(tc.tile_pool(name="pwqs", bufs=1))
        pe_ = s1.enter_context(tc.tile_pool(name="pe", bufs=12))
        pstg = s1.enter_context(tc.tile_pool(name="pstg", bufs=2))
        prc = s1.enter_context(tc.tile_pool(name="prc", bufs=6))
        pp1 = s1.enter_context(tc.tile_pool(name="pp1", bufs=1))
        pdn = s1.enter_context(tc.tile_pool(name="pdn", bufs=1))
        pfin = s1.enter_context(tc.tile_pool(name="pfin", bufs=12))
        psA = s1.enter_context(tc.tile_pool(name="psA", bufs=2, space="PSUM"))
        psO = s1.enter_context(tc.tile_pool(name="psO", bufs=2, space="PSUM"))
        psP = s1.enter_context(tc.tile_pool(name="psP", bufs=2, space="PSUM"))

        # ---- startup loads ----
        # DMA queue order matters: per-queue, transfers land in issue order,
        # and the DMA engines drain roughly in global issue order.
        wqt = [
            pwqs.tile([128, KC * 128], BF16, tag=f"wq{m}", name=f"wq{m}")
            for m in range(2 * KC)
        ]
        w_row = pdn.tile([1, N], F32, tag="dn", name="w_row")
        nc.scalar.dma_start(w_row[:], w_d[:])
        wcol = const.tile([128, NT], F32, tag="wcol")
        nc.sync.dma_start(wcol[:], w_d[0, :].rearrange("(r p) -> p r", p=128))
        nc.scalar.dma_start(wqt[0][:], wqk_d[0:128, :])
        xtt = [pxt.tile([128, N], BF16, tag=f"xt{c}", name=f"xt{c}") for c in range(KC)]
        # x^T loads split across both HWDGE queues, landing in c order
        nc.sync.dma_start(xtt[0][:, 0:512], xt_d[0:128, 0:512])
        nc.sync.dma_start(xtt[0][:, 512:1024], xt_d[0:128, 512:1024])
        nc.scalar.dma_start(wqt[KC][:], wqk_d[ts(KC, 128), :])
        for c in range(1, KC):
            nc.sync.dma_start(xtt[c][:, 0:512], xt_d[ts(c, 128), 0:512])
            nc.scalar.dma_start(xtt[c][:, 512:1024], xt_d[ts(c, 128), 512:1024])
        wvt = [pwv.tile([128, D], BF16, tag=f"wv{c}", name=f"wv{c}") for c in range(KC)]
        for c in range(KC):
            eng = nc.sync if c % 2 == 0 else nc.scalar
            eng.dma_start(wvt[c][:], wv_d[ts(c, 128), :])
        # remaining qk weight chunks, in consumption order
        for i, m in enumerate([1, KC + 1, 2, KC + 2, 3, KC + 3, 4, KC + 4, 5, KC + 5]):
            eng = nc.sync if i % 2 == 0 else nc.scalar
            eng.dma_start(wqt[m][:], wqk_d[ts(m, 128), :])
        bias = const.tile([128, KC], F32, tag="bias")
        nc.sync.dma_start(bias[:], bm_d[:].rearrange("(c p) -> p c", p=128))
        ident = const.tile([128, 128], BF16, tag="ident")
        nc.scalar.dma_start(ident[:], id_d[:])

        ones_bf = const.tile([128, HD], BF16, tag="ones_bf")
        nc.vector.memset(ones_bf[:], 1.0)
        jconst = const.tile([128, 512], BF16, tag="jconst")
        nc.vector.memset(jconst[:], 0.5)

        # ---- PE warm-up: the clock gate holds PE at reduced speed until
        # ~3us of sustained activity; junk matmuls on resident const tiles
        # cover the initial DMA wait so real work starts at full clock.
        psj = psA.tile([128, N], F32, tag="psA", name="psj")
        for _ in range(2):
            nc.tensor.matmul(
                psj[0:HD, 0:HD], ones_bf[:], ones_bf[:], start=True, stop=True
            )
        for _ in range(9):
            nc.tensor.matmul(
                psj[0:HD, 0:512], ones_bf[:], jconst[:], start=True, stop=True
            )

        wb = const.tile([128, N], F32, tag="wb")
        nc.gpsimd.partition_broadcast(wb[:], w_row[:])
        # per-key token gate folded into the exp scale (per-partition AP)
        wcolS = const.tile([128, NT], F32, tag="wcolS")
        nc.vector.tensor_scalar_mul(wcolS[:], wcol[:], SCALE)

        qkt = [pqk.tile([128, N], BF16, tag=f"qk{m}", name=f"qk{m}") for m in range(2 * KC)]
        vt = [pv.tile([128, H * HE], BF16, tag=f"v{r}", name=f"v{r}") for r in range(NT)]
        ott = [pot.tile([128, N], BF16, tag=f"ot{c}", name=f"ot{c}") for c in range(KC)]
        wmt = [pwm.tile([128, D], BF16, tag=f"wm{c}", name=f"wm{c}") for c in range(KC)]
        p1t = [
            pp1.tile([128, 512], BF16, tag=f"p1_{g}", name=f"p1_{g}")
            for g in range(2 * KC)
        ]

        # shared state between the main S/exp loop and the filler gens
        e_store: dict = {}   # (h, r) -> e tile
        o_ps: dict = {}      # h -> (oa, ob)
        stg_cur: dict = {}   # qc -> pair staging tile
        stg5: dict = {}      # qc -> pair-5 staging tile

        def gen_qk(m, pool=None):
            """qk^T chunk m. Yields (per contraction step) its PE cost.

            q chunks (m < KC) fold the token gate in via a DVE multiply;
            k chunks copy plainly on ACT -- their gate rides in the exp
            scale argument (per-partition = per-key)."""
            wq_m = wqt[m]
            pool = pool or psP
            tag = "psP" if pool is psP else "psO"
            ps0 = pool.tile([128, 512], F32, tag=tag, name="psP")
            ps1 = pool.tile([128, 512], F32, tag=tag, name="psP")
            pj = (ps0, ps1)
            for c in range(KC):
                for j in range(NC2):
                    nc.tensor.matmul(
                        pj[j][:],
                        wq_m[:, ts(c, 128)],
                        xtt[c][:, ts(j, 512)],
                        start=(c == 0),
                        stop=(c == KC - 1),
                    )
                yield 427
            yield 0
            for j in range(NC2):
                nc.vector.tensor_mul(
                    qkt[m][:, ts(j, 512)], pj[j][:], wb[:, ts(j, 512)]
                )

        def gen_v():
            """V projection, fine-grained: yields per (r, c) step."""
            for r in range(NT):
                ta = psP.tile([128, 512], F32, tag="psP", name="psP")
                tb = psP.tile([128, 512], F32, tag="psP", name="psP")
                for c in range(KC):
                    nc.tensor.matmul(
                        ta[:],
                        xtt[c][:, ts(r, 128)],
                        wvt[c][:, 0:512],
                        start=(c == 0),
                        stop=(c == KC - 1),
                    )
                    nc.tensor.matmul(
                        tb[:, 0:256],
                        xtt[c][:, ts(r, 128)],
                        wvt[c][:, 512:768],
                        start=(c == 0),
                        stop=(c == KC - 1),
                    )
                    yield 320
                v3 = vt[r][:].rearrange("p (h e) -> p h e", e=HE)
                nc.vector.tensor_copy(
                    v3[:, :, HD : HD + 1],
                    ones_bf[:, 0:H].rearrange("p (h o) -> p h o", o=1),
                )
                nc.vector.tensor_scalar_mul(
                    v3[:, 0:8, 0:HD],
                    ta[:].rearrange("p (h e) -> p h e", e=HD),
                    wcol[:, r : r + 1],
                )
                nc.vector.tensor_scalar_mul(
                    v3[:, 8:12, 0:HD],
                    tb[:, 0:256].rearrange("p (h e) -> p h e", e=HD),
                    wcol[:, r : r + 1],
                )
                yield 50

        def gen_o(h):
            """A@V accumulation steps for head h; one yield per r-chunk."""
            oa = psO.tile([128, 512], F32, tag="psO", name="psO")
            ob = psO.tile([128, 512], F32, tag="psO", name="psO")
            o_ps[h] = (oa, ob)
            for r in range(NT):
                while (h, r) not in e_store:
                    yield BLOCK
                e = e_store.pop((h, r))
                for qc in range(NT):
                    po = oa if qc < 4 else ob
                    off = HE * (qc % 4)
                    nc.tensor.matmul(
                        po[:, off : off + HE],
                        e[:, ts(qc, 128)],
                        vt[r][:, h * HE : (h + 1) * HE],
                        start=(r == 0 and qc % 4 == 0),
                        stop=(r == NT - 1),
                    )
                yield 217

        def gen_norm(h):
            """normalize head h into pair staging; odd heads also emit the
            pair's XBAR transposes into ott. One yield per token chunk."""
            oa, ob = o_ps.pop(h)
            for qc in range(NT):
                po = oa if qc < 4 else ob
                off = HE * (qc % 4)
                rc = prc.tile([128, 1], F32, tag="rc", name="rc")
                nc.vector.reciprocal_approx_fast(
                    rc[:], po[:, off + HD : off + HD + 1]
                )
                stg = stg_cur
                if h >= 10:
                    stg = stg5
                if h % 2 == 0:
                    stg[qc] = pstg.tile(
                        [128, 128], BF16, tag=f"stg{qc}", name=f"stg{qc}"
                    )
                nc.vector.tensor_scalar_mul(
                    stg[qc][:, HD * (h % 2) : HD * (h % 2) + HD],
                    po[:, off : off + HD],
                    rc[:, 0:1],
                )
                if h % 2 == 1 and h < 10:
                    nc.sync.dma_start_transpose(
                        ott[h // 2][:, ts(qc, 128)], stg[qc][:]
                    )
                yield 60

        def gen_stg5_transposes():
            """head-10 halves of pair 5 ride to ott[5] rows 0:64 (cols 64:128
            carry stale data; the tail rewrites rows 64:128)."""
            for qc in range(NT):
                nc.sync.dma_start_transpose(ott[KC - 1][:, ts(qc, 128)], stg5[qc][:])
                yield 60

        def gen_p1(c, j):
            """output-projection partial k=0..4 for group (c, j), in two
            lumps; the copy folds the bias in and frees the bank."""
            ps = psP.tile([128, 512], F32, tag="psP", name="psP")
            for k in range(3):
                nc.tensor.matmul(
                    ps[:],
                    wmt[k][:, ts(c, 128)],
                    ott[k][:, ts(j, 512)],
                    start=(k == 0),
                    stop=False,
                )
            yield 640
            for k in range(3, KC - 1):
                nc.tensor.matmul(
                    ps[:],
                    wmt[k][:, ts(c, 128)],
                    ott[k][:, ts(j, 512)],
                    start=False,
                    stop=(k == KC - 2),
                )
            nc.vector.tensor_scalar_add(p1t[2 * c + j][:], ps[:], bias[:, c : c + 1])
            yield 427

        def gen_wmt_loads():
            for c in range(KC):
                nc.sync.dma_start(wmt[c][:], wm_d[ts(c, 128), :])
            yield 0

        # ---- ordered filler chain ----
        def filler_chain():
            yield from gen_qk(1)
            yield from gen_qk(KC + 1)
            yield from gen_v()
            yield from gen_wmt_loads()
            yield from gen_o(0)
            yield from gen_norm(0)
            yield from gen_qk(2)
            yield from gen_o(1)
            yield from gen_norm(1)
            yield from gen_qk(KC + 2)
            yield from gen_o(2)
            yield from gen_norm(2)
            yield from gen_qk(3)
            yield from gen_o(3)
            yield from gen_norm(3)
            yield from gen_qk(KC + 3)
            yield from gen_o(4)
            yield from gen_norm(4)
            yield from gen_qk(4)
            yield from gen_o(5)
            yield from gen_norm(5)
            yield from gen_qk(KC + 4)
            yield from gen_o(6)
            yield from gen_norm(6)
            yield from gen_qk(5)
            yield from gen_o(7)
            yield from gen_norm(7)
            yield from gen_qk(KC + 5)
            yield from gen_o(8)
            yield from gen_norm(8)
            yield from gen_o(9)
            yield from gen_norm(9)
            for c in range(KC):
                yield from gen_p1(c, 0)
            yield from gen_o(10)
            yield from gen_norm(10)
            yield from gen_stg5_transposes()
            for c in range(KC):
                yield from gen_p1(c, 1)

        fill_src = filler_chain()
        owed = [0.0]

        def fill(ns):
            owed[0] += ns
            while owed[0] > 0:
                c = next(fill_src, None)
                if c is None:
                    owed[0] = 0
                    return
                if c >= BLOCK:
                    # filler is waiting on the main loop; retry next slot
                    owed[0] = 0
                    return
                owed[0] -= max(c, 40)

        # per-head filler budget (ns of planned PE work per r-slot)
        budgets = dict(enumerate(BUDGETS))

        # ---- main S/exp loop ----
        # chunks 0 and 6 interleave c-steps so PE tracks the x^T arrivals
        for _steps in zip_longest(gen_qk(0), gen_qk(KC, pool=psO)):
            pass
        # bridge the qkt gate-copy latency with junk so the PE clock ramp
        # never resets before the first S matmul
        psj2 = psA.tile([128, N], F32, tag="psA", name="psj2")
        for _ in range(BRIDGE):
            nc.tensor.matmul(
                psj2[0:HD, 0:512], ones_bf[:], jconst[:], start=True, stop=True
            )

        for h in range(H):
            qt, qr = qkt[h // 2], HD * (h % 2)
            kt = qkt[KC + h // 2]
            for r in range(NT):
                ps = psA.tile([128, N], F32, tag="psA", name="psA")
                for j in range(NC2):
                    nc.tensor.matmul(
                        ps[:, ts(j, 512)],
                        kt[qr : qr + HD, ts(r, 128)],
                        qt[qr : qr + HD, ts(j, 512)],
                        start=True,
                        stop=True,
                    )
                e = pe_.tile([128, N], BF16, tag="e", name="e")
                nc.scalar.activation(e[:], ps[:], AF.Exp, scale=SCALE)
                e_store[(h, r)] = e
                fill(budgets[h])

        # ---- tail: head-11 O + norm + PE transposes + final projection ----
        fill(10**9)  # drain any remaining filler work
        oa = psO.tile([128, 512], F32, tag="psO", name="psO")
        ob = psO.tile([128, 512], F32, tag="psO", name="psO")
        for r in range(NT):
            e = e_store.pop((11, r))
            for qc in range(NT):
                po = oa if qc < 4 else ob
                off = HE * (qc % 4)
                nc.tensor.matmul(
                    po[:, off : off + HE],
                    e[:, ts(qc, 128)],
                    vt[r][:, 11 * HE : 12 * HE],
                    start=(r == 0 and qc % 4 == 0),
                    stop=(r == NT - 1),
                )
        # pre-inject the first two partials while the norm chain runs
        psl = [psP.tile([128, 512], F32, tag="psP", name="psF")[:] for _ in range(2)]
        for g in range(2):
            nc.tensor.matmul(psl[g], ident[:], p1t[2 * g][:], start=True, stop=False)
        trt = None
        for qc in range(NT):
            po = oa if qc < 4 else ob
            off = HE * (qc % 4)
            rc = prc.tile([128, 1], F32, tag="rc", name="rc")
            nc.vector.reciprocal_approx_fast(rc[:], po[:, off + HD : off + HD + 1])
            s11 = pstg.tile([128, HD], BF16, tag="s11", name="s11", bufs=4)
            nc.scalar.activation(
                s11[:], po[:, off : off + HD], AF.Copy, scale=rc[:, 0:1]
            )
            if qc % 2 == 0:
                trt = psA.tile([128, N], F32, tag="psA", name="psA")
            dst = trt[0:HD, 512 * (qc % 2) : 512 * (qc % 2) + HD].bitcast(BF16)
            nc.tensor.transpose(dst, s11[:], ident[:])
            nc.vector.tensor_copy(ott[KC - 1][HD:128, ts(qc, 128)], dst)
        # final projection step: re-inject partials, add k=5, copy, store.
        # Groups rotate across psP(2) + psO(2) + two psA tiles (4 banks).
        trp = [psA.tile([128, N], F32, tag="psA", name="psA") for _ in range(2)]
        slots = (
            psl
            + [psO.tile([128, 512], F32, tag="psO", name="psF")[:] for _ in range(2)]
            + [t[:, ts(j, 512)] for t in trp for j in range(2)]
        )
        g = 0
        for j in range(NC2):
            for c in range(KC):
                ps = slots[g % 8]
                if g >= 2:
                    nc.tensor.matmul(
                        ps, ident[:], p1t[2 * c + j][:], start=True, stop=False
                    )
                nc.tensor.matmul(
                    ps,
                    wmt[KC - 1][:, ts(c, 128)],
                    ott[KC - 1][:, ts(j, 512)],
                    start=False,
                    stop=True,
                )
                fin = pfin.tile([128, 512], BF16, tag="fin", name="fin")
                ha, hb = (0, 256) if g % 2 == 0 else (256, 0)
                nc.scalar.activation(fin[:, ha : ha + 256], ps[:, ha : ha + 256], AF.Copy)
                nc.vector.tensor_copy(fin[:, hb : hb + 256], ps[:, hb : hb + 256])
                eng = nc.sync if g % 2 == 0 else nc.scalar
                eng.dma_start(y_d[ts(c, 128), ts(j, 512)], fin[:])
                g += 1


def _build(repeat=1):
    key = ("nc", repeat)
    if key not in _CACHE:
        nc = bacc.Bacc("TRN2", target_bir_lowering=False, debug=False, num_devices=B)
        with tile.TileContext(nc) as tc:
            _emit(tc, repeat=repeat)
        nc.compile()
        _CACHE[key] = nc
    return _CACHE[key]


def kernel(x, weight, W_qkv, W_msa, b_msa):
    nc = _build()
    x = np.asarray(x, dtype=np.float32)
    weight = np.asarray(weight, dtype=np.float32)
    W_qkv = np.asarray(W_qkv, dtype=np.float32)
    bf = ml_dtypes.bfloat16
    # pack [q;k]^T chunk-major: row block m = the six [128,128] stationary
    # pieces of qk chunk m side by side (see _emit)
    wqk = np.ascontiguousarray(
        W_qkv[:, : 2 * D]
        .reshape(KC, 128, 2 * KC, 128)
        .transpose(2, 1, 0, 3)
        .reshape(2 * D, D)
    ).astype(bf)
    wv = np.ascontiguousarray(W_qkv[:, 2 * D :]).astype(bf)
    wm = np.asarray(W_msa, dtype=np.float32).astype(bf)
    ident = np.eye(128, dtype=np.float32).astype(bf)
    in_maps = []
    for b in range(B):
        in_maps.append(
            {
                "xt": np.ascontiguousarray(x[b].T).astype(bf),
                "w": np.ascontiguousarray(weight[b : b + 1]),
                "wqk": wqk,
                "wv": wv,
                "wmsa": wm,
                "bmsa": np.asarray(b_msa, dtype=np.float32),
                "ident": ident,
            }
        )
    res = run_bass_kernel_spmd(nc, in_maps, list(range(B)))
    out = np.stack(
        [res.results[b]["yt"].astype(np.float32).T for b in range(B)], axis=0
    )
    return np.ascontiguousarray(out)


# revision 59
# speedup vs baseline: 1.0637x; 1.0123x over previous
"""Trainium2 Bass kernel for nn_Attention_35923106463893.

Multi-head attention block:
    qkv = (weight[:, :, None] * (x @ W_qkv)) -> split q,k,v over 12 heads
    A = softmax(q k^T / sqrt(64));  out = (A v) reshaped @ W_msa + b_msa

Sharding: pure data-parallel over batch B=8 -> one batch element per
NeuronCore, no collectives.

Structure (all matmul operands bf16; PSUM accumulation fp32):
- qk^T computed in transposed layout (head-dims on partitions) as 12
  [128, N] chunks; token gate folded into the PSUM->SBUF copies.
- Per-head scores S^T [keys, queries] -> ACT exp -> E bf16.
- A@V runs re-oriented: out[q, 65] accumulates E^T-stationary x [V|1]
  moving (65-row matmuls are charged by moving size only), so the 12
  heads cost ~21us instead of ~41us on PE. The ones column of V makes
  psum col 64 the softmax denominator; normalization is a per-partition
  reciprocal + scalar-mul.
- Normalized attention transposes back to [d, tokens] for the output
  projection via DMA-XBAR transpose instructions (off the compute
  engines). The final head pair transposes via PE at the tail.
- Output projection y^T = W_msa^T attn^T accumulates per (c, j) in
  PSUM; k=0..4 partials run during heads 10-11 and are copied (bias
  folded in) to SBUF, so after head 11 only an identity re-inject
  + k=5 step + copy + bf16 DMA remain.

Scheduling: the main loop emits S/exp per (head, r-chunk) slot; all
other PE work (V projection, O accumulation steps, qk chunks,
projection partials, normalizations) lives in one ordered filler chain
whose generators yield their PE cost in ns. Each slot consumes filler
work against a per-head budget, keeping the in-order PE stream dense
without head-of-line blocking.
"""

from contextlib import ExitStack
from itertools import zip_longest

import numpy as np
import ml_dtypes

import concourse.bass as bass
import concourse.mybir as mybir
import concourse.tile as tile
from concourse import bacc
from concourse.bass import ts
from concourse.bass_utils import run_bass_kernel_spmd

B, N, D, H = 8, 1024, 768, 12
HD = D // H          # 64
HE = HD + 1          # 65: head value dims + ones column
SCALE = HD ** -0.5   # 0.125
KC = D // 128        # 6 contraction chunks
NT = N // 128        # 8 token chunks
NC2 = N // 512       # 2 moving chunks

F32 = mybir.dt.float32
F32R = mybir.dt.float32r
BF16 = mybir.dt.bfloat16
AF = mybir.ActivationFunctionType

_CACHE: dict = {}
BLOCK = 10**9  # filler-chain sentinel: "waiting on the main loop"
# per-head filler budgets (ns of planned PE work per r-slot), tuned on the
# timeline model
BUDGETS = [700, 1250, 1250, 700, 650, 650, 650, 650, 650, 800, 1300, 1300]
BRIDGE = 6  # junk matmuls bridging the first-S gate-copy latency
DEBT_CAP = 1500  # max carried filler budget across a blocked slot (ns)


def _run(gen):
    for _ in gen:
        pass


def _emit(tc, repeat=1):
    nc = tc.nc
    xt_d = nc.dram_tensor("xt", [D, N], BF16, kind="ExternalInput").ap()
    w_d = nc.dram_tensor("w", [1, N], F32, kind="ExternalInput").ap()
    # chunk-major packed [q;k]^T weights: row block m holds the six
    # [128, 128] stationary pieces of qk chunk m side by side
    wqk_d = nc.dram_tensor("wqk", [2 * D, D], BF16, kind="ExternalInput").ap()
    wv_d = nc.dram_tensor("wv", [D, D], BF16, kind="ExternalInput").ap()
    wm_d = nc.dram_tensor("wmsa", [D, D], BF16, kind="ExternalInput").ap()
    bm_d = nc.dram_tensor("bmsa", [D], F32, kind="ExternalInput").ap()
    id_d = nc.dram_tensor("ident", [128, 128], BF16, kind="ExternalInput").ap()
    y_d = nc.dram_tensor("yt", [D, N], BF16, kind="ExternalOutput").ap()

    for _rep in range(repeat):
        _emit_body(tc, xt_d, w_d, wqk_d, wv_d, wm_d, bm_d, id_d, y_d)


def _emit_body(tc, xt_d, w_d, wqk_d, wv_d, wm_d, bm_d, id_d, y_d):
    nc = tc.nc
    with ExitStack() as s1:
        const = s1.enter_context(tc.tile_pool(name="const", bufs=1))
        pwm = s1.enter_context(tc.tile_pool(name="pwm", bufs=1))
        pwv = s1.enter_context(tc.tile_pool(name="pwv", bufs=1))
        pqk = s1.enter_context(tc.tile_pool(name="pqk", bufs=1))
        pv = s1.enter_context(tc.tile_pool(name="pv", bufs=1))
        pot = s1.enter_context(tc.tile_pool(name="pot", bufs=1))
        pxt = s1.enter_context(tc.tile_pool(name="pxt", bufs=1))
        pwqs = s1.enter_context(tc.tile_pool(name="pwqs", bufs=1))
        pe_ = s1.enter_context(tc.tile_pool(name="pe", bufs=12))
        pstg = s1.enter_context(tc.tile_pool(name="pstg", bufs=2))
        prc = s1.enter_context(tc.tile_pool(name="prc", bufs=6))
        pp1 = s1.enter_context(tc.tile_pool(name="pp1", bufs=1))
        pdn = s1.enter_context(tc.tile_pool(name="pdn", bufs=1))
        pfin = s1.enter_context(tc.tile_pool(name="pfin", bufs=12))
        psA = s1.enter_context(tc.tile_pool(name="psA", bufs=2, space="PSUM"))
        psO = s1.enter_context(tc.tile_pool(name="psO", bufs=2, space="PSUM"))
        psP = s1.enter_context(tc.tile_pool(name="psP", bufs=2, space="PSUM"))

        # ---- startup loads ----
        # DMA queue order matters: per-queue, transfers land in issue order,
        # and the DMA engines drain roughly in global issue order.
        wqt = [
            pwqs.tile([128, KC * 128], BF16, tag=f"wq{m}", name=f"wq{m}")
            for m in range(2 * KC)
        ]
        w_row = pdn.tile([1, N], F32, tag="dn", name="w_row")
        nc.scalar.dma_start(w_row[:], w_d[:])
        wcol = const.tile([128, NT], F32, tag="wcol")
        nc.sync.dma_start(wcol[:], w_d[0, :].rearrange("(r p) -> p r", p=128))
        nc.scalar.dma_start(wqt[0][:], wqk_d[0:128, :])
        xtt = [pxt.tile([128, N], BF16, tag=f"xt{c}", name=f"xt{c}") for c in range(KC)]
        # x^T loads split across both HWDGE queues, landing in c order
        nc.sync.dma_start(xtt[0][:, 0:512], xt_d[0:128, 0:512])
        nc.sync.dma_start(xtt[0][:, 512:1024], xt_d[0:128, 512:1024])
        nc.scalar.dma_start(wqt[KC][:], wqk_d[ts(KC, 128), :])
        for c in range(1, KC):
            nc.sync.dma_start(xtt[c][:, 0:512], xt_d[ts(c, 128), 0:512])
            nc.scalar.dma_start(xtt[c][:, 512:1024], xt_d[ts(c, 128), 512:1024])
        wvt = [pwv.tile([128, D], BF16, tag=f"wv{c}", name=f"wv{c}") for c in range(KC)]
        for c in range(KC):
            eng = nc.sync if c % 2 == 0 else nc.scalar
            eng.dma_start(wvt[c][:], wv_d[ts(c, 128), :])
        # remaining qk weight chunks, in consumption order
        for i, m in enumerate([1, KC + 1, 2, KC + 2, 3, KC + 3, 4, KC + 4, 5, KC + 5]):
            eng = nc.sync if i % 2 == 0 else nc.scalar
            eng.dma_start(wqt[m][:], wqk_d[ts(m, 128), :])
        bias = const.tile([128, KC], F32, tag="bias")
        nc.sync.dma_start(bias[:], bm_d[:].rearrange("(c p) -> p c", p=128))
        ident = const.tile([128, 128], BF16, tag="ident")
        nc.scalar.dma_start(ident[:], id_d[:])

        ones_bf = const.tile([128, HD], BF16, tag="ones_bf")
        nc.vector.memset(ones_bf[:], 1.0)
        jconst = const.tile([128, 512], BF16, tag="jconst")
        nc.vector.memset(jconst[:], 0.5)

        # ---- PE warm-up: the clock gate holds PE at reduced speed until
        # ~3us of sustained activity; junk matmuls on resident const tiles
        # cover the initial DMA wait so real work starts at full clock.
        psj = psA.tile([128, N], F32, tag="psA", name="psj")
        for _ in range(2):
            nc.tensor.matmul(
                psj[0:HD, 0:HD], ones_bf[:], ones_bf[:], start=True, stop=True
            )
        for _ in range(9):
            nc.tensor.matmul(
                psj[0:HD, 0:512], ones_bf[:], jconst[:], start=True, stop=True
            )

        wb = const.tile([128, N], F32, tag="wb")
        nc.gpsimd.partition_broadcast(wb[:], w_row[:])
        # per-key token gate folded into the exp scale (per-partition AP)
        wcolS = const.tile([128, NT], F32, tag="wcolS")
        nc.vector.tensor_scalar_mul(wcolS[:], wcol[:], SCALE)

        qkt = [pqk.tile([128, N], BF16, tag=f"qk{m}", name=f"qk{m}") for m in range(2 * KC)]
        vt = [pv.tile([128, H * HE], BF16, tag=f"v{r}", name=f"v{r}") for r in range(NT)]
        ott = [pot.tile([128, N], BF16, tag=f"ot{c}", name=f"ot{c}") for c in range(KC)]
        wmt = [pwm.tile([128, D], BF16, tag=f"wm{c}", name=f"wm{c}") for c in range(KC)]
        p1t = [
            pp1.tile([128, 512], BF16, tag=f"p1_{g}", name=f"p1_{g}")
            for g in range(2 * KC)
        ]

        # shared state between the main S/exp loop and the filler gens
        e_store: dict = {}   # (h, r) -> e tile
        o_ps: dict = {}      # h -> (oa, ob)
        stg_cur: dict = {}   # qc -> pair staging tile
        stg5: dict = {}      # qc -> pair-5 staging tile

        def gen_qk(m, pool=None):
            """qk^T chunk m. Yields (per contraction step) its PE cost.

            q chunks (m < KC) fold the token gate in via a DVE multiply;
            k chunks copy plainly on ACT -- their gate rides in the exp
            scale argument (per-partition = per-key)."""
            wq_m = wqt[m]
            pool = pool or psP
            tag = "psP" if pool is psP else "psO"
            ps0 = pool.tile([128, 512], F32, tag=tag, name="psP")
            ps1 = pool.tile([128, 512], F32, tag=tag, name="psP")
            pj = (ps0, ps1)
            for c in range(KC):
                for j in range(NC2):
                    nc.tensor.matmul(
                        pj[j][:],
                        wq_m[:, ts(c, 128)],
                        xtt[c][:, ts(j, 512)],
                        start=(c == 0),
                        stop=(c == KC - 1),
                    )
                yield 427
            yield 0
            for j in range(NC2):
                nc.vector.tensor_mul(
                    qkt[m][:, ts(j, 512)], pj[j][:], wb[:, ts(j, 512)]
                )

        def gen_v():
            """V projection, fine-grained: yields per (r, c) step."""
            for r in range(NT):
                ta = psP.tile([128, 512], F32, tag="psP", name="psP")
                tb = psP.tile([128, 512], F32, tag="psP", name="psP")
                for c in range(KC):
                    nc.tensor.matmul(
                        ta[:],
                        xtt[c][:, ts(r, 128)],
                        wvt[c][:, 0:512],
                        start=(c == 0),
                        stop=(c == KC - 1),
                    )
                    nc.tensor.matmul(
                        tb[:, 0:256],
                        xtt[c][:, ts(r, 128)],
                        wvt[c][:, 512:768],
                        start=(c == 0),
                        stop=(c == KC - 1),
                    )
                    yield 320
                v3 = vt[r][:].rearrange("p (h e) -> p h e", e=HE)
                nc.vector.tensor_copy(
                    v3[:, :, HD : HD + 1],
                    ones_bf[:, 0:H].rearrange("p (h o) -> p h o", o=1),
                )
                nc.vector.tensor_scalar_mul(
                    v3[:, 0:8, 0:HD],
                    ta[:].rearrange("p (h e) -> p h e", e=HD),
                    wcol[:, r : r + 1],
                )
                nc.vector.tensor_scalar_mul(
                    v3[:, 8:12, 0:HD],
                    tb[:, 0:256].rearrange("p (h e) -> p h e", e=HD),
                    wcol[:, r : r + 1],
                )
                yield 50

        def gen_o(h):
            """A@V accumulation steps for head h; one yield per r-chunk."""
            oa = psO.tile([128, 512], F32, tag="psO", name="psO")
            ob = psO.tile([128, 512], F32, tag="psO", name="psO")
            o_ps[h] = (oa, ob)
            for r in range(NT):
                while (h, r) not in e_store:
                    yield BLOCK
                e = e_store.pop((h, r))
                for qc in range(NT):
                    po = oa if qc < 4 else ob
                    off = HE * (qc % 4)
                    nc.tensor.matmul(
                        po[:, off : off + HE],
                        e[:, ts(qc, 128)],
                        vt[r][:, h * HE : (h + 1) * HE],
                        start=(r == 0 and qc % 4 == 0),
                        stop=(r == NT - 1),
                    )
                yield 217

        def gen_norm(h):
            """normalize head h into pair staging; odd heads also emit the
            pair's XBAR transposes into ott. One yield per token chunk."""
            oa, ob = o_ps.pop(h)
            for qc in range(NT):
                po = oa if qc < 4 else ob
                off = HE * (qc % 4)
                rc = prc.tile([128, 1], F32, tag="rc", name="rc")
                nc.vector.reciprocal_approx_fast(
                    rc[:], po[:, off + HD : off + HD + 1]
                )
                stg = stg_cur
                if h >= 10:
                    stg = stg5
                if h % 2 == 0:
                    stg[qc] = pstg.tile(
                        [128, 128], BF16, tag=f"stg{qc}", name=f"stg{qc}"
                    )
                nc.vector.tensor_scalar_mul(
                    stg[qc][:, HD * (h % 2) : HD * (h % 2) + HD],
                    po[:, off : off + HD],
                    rc[:, 0:1],
                )
                if h % 2 == 1 and h < 10:
                    nc.sync.dma_start_transpose(
                        ott[h // 2][:, ts(qc, 128)], stg[qc][:]
                    )
                yield 60

        def gen_stg5_transposes():
            """head-10 halves of pair 5 ride to ott[5] rows 0:64 (cols 64:128
            carry stale data; the tail rewrites rows 64:128)."""
            for qc in range(NT):
                nc.sync.dma_start_transpose(ott[KC - 1][:, ts(qc, 128)], stg5[qc][:])
                yield 60

        def gen_p1(c, j):
            """output-projection partial k=0..4 for group (c, j), in two
            lumps; the copy folds the bias in and frees the bank."""
            ps = psP.tile([128, 512], F32, tag="psP", name="psP")
            for k in range(3):
                nc.tensor.matmul(
                    ps[:],
                    wmt[k][:, ts(c, 128)],
                    ott[k][:, ts(j, 512)],
                    start=(k == 0),
                    stop=False,
                )
            yield 640
            for k in range(3, KC - 1):
                nc.tensor.matmul(
                    ps[:],
                    wmt[k][:, ts(c, 128)],
                    ott[k][:, ts(j, 512)],
                    start=False,
                    stop=(k == KC - 2),
                )
            nc.vector.tensor_scalar_add(p1t[2 * c + j][:], ps[:], bias[:, c : c + 1])
            yield 427

        def gen_wmt_loads():
            for c in range(KC):
                nc.sync.dma_start(wmt[c][:], wm_d[ts(c, 128), :])
            yield 0

        # ---- ordered filler chain ----
        def filler_chain():
            yield from gen_qk(1)
            yield from gen_qk(KC + 1)
            yield from gen_v()
            yield from gen_wmt_loads()
            yield from gen_o(0)
            yield from gen_norm(0)
            yield from gen_qk(2)
            yield from gen_o(1)
            yield from gen_norm(1)
            yield from gen_qk(KC + 2)
            yield from gen_o(2)
            yield from gen_norm(2)
            yield from gen_qk(3)
            yield from gen_o(3)
            yield from gen_norm(3)
            yield from gen_qk(KC + 3)
            yield from gen_o(4)
            yield from gen_norm(4)
            yield from gen_qk(4)
            yield from gen_o(5)
            yield from gen_norm(5)
            yield from gen_qk(KC + 4)
            yield from gen_o(6)
            yield from gen_norm(6)
            yield from gen_qk(5)
            yield from gen_o(7)
            yield from gen_norm(7)
            yield from gen_qk(KC + 5)
            yield from gen_o(8)
            yield from gen_norm(8)
            yield from gen_o(9)
            yield from gen_norm(9)
            for c in range(KC):
                yield from gen_p1(c, 0)
            yield from gen_o(10)
            yield from gen_norm(10)
            yield from gen_stg5_transposes()
            for c in range(KC):
                yield from gen_p1(c, 1)

        fill_src = filler_chain()
        owed = [0.0]

        def fill(ns):
            owed[0] += ns
            while owed[0] > 0:
                c = next(fill_src, None)
                if c is None:
                    owed[0] = 0
                    return
                if c >= BLOCK:
                    # filler is waiting on the main loop; retry next slot,
                    # carrying (bounded) unspent budget as debt
                    owed[0] = min(owed[0], DEBT_CAP)
                    return
                owed[0] -= max(c, 40)

        # per-head filler budget (ns of planned PE work per r-slot)
        budgets = dict(enumerate(BUDGETS))

        # ---- main S/exp loop ----
        # chunks 0 and 6 interleave c-steps so PE tracks the x^T arrivals
        for _steps in zip_longest(gen_qk(0), gen_qk(KC, pool=psO)):
            pass
        # bridge the qkt gate-copy latency with junk so the PE clock ramp
        # never resets before the first S matmul
        psj2 = psA.tile([128, N], F32, tag="psA", name="psj2")
        for _ in range(BRIDGE):
            nc.tensor.matmul(
                psj2[0:HD, 0:512], ones_bf[:], jconst[:], start=True, stop=True
            )

        for h in range(H):
            qt, qr = qkt[h // 2], HD * (h % 2)
            kt = qkt[KC + h // 2]
            for r in range(NT):
                ps = psA.tile([128, N], F32, tag="psA", name="psA")
                for j in range(NC2):
                    nc.tensor.matmul(
                        ps[:, ts(j, 512)],
                        kt[qr : qr + HD, ts(r, 128)],
                        qt[qr : qr + HD, ts(j, 512)],
                        start=True,
                        stop=True,
                    )
                e = pe_.tile([128, N], BF16, tag="e", name="e")
                nc.scalar.activation(e[:], ps[:], AF.Exp, scale=SCALE)
                e_store[(h, r)] = e
                fill(budgets[h])

        # ---- tail: head-11 O + norm + PE transposes + final projection ----
        fill(10**9)  # drain any remaining filler work
        oa = psO.tile([128, 512], F32, tag="psO", name="psO")
        ob = psO.tile([128, 512], F32, tag="psO", name="psO")
        for r in range(NT):
            e = e_store.pop((11, r))
            for qc in range(NT):
                po = oa if qc < 4 else ob
                off = HE * (qc % 4)
                nc.tensor.matmul(
                    po[:, off : off + HE],
                    e[:, ts(qc, 128)],
                    vt[r][:, 11 * HE : 12 * HE],
                    start=(r == 0 and qc % 4 == 0),
                    stop=(r == NT - 1),
                )
        # pre-inject the first two partials while the norm chain runs
        psl = [psP.tile([128, 512], F32, tag="psP", name="psF")[:] for _ in range(2)]
        for g in range(2):
            nc.tensor.matmul(psl[g], ident[:], p1t[2 * g][:], start=True, stop=False)
        trt = None
        for qc in range(NT):
            po = oa if qc < 4 else ob
            off = HE * (qc % 4)
            rc = prc.tile([128, 1], F32, tag="rc", name="rc")
            nc.vector.reciprocal_approx_fast(rc[:], po[:, off + HD : off + HD + 1])
            s11 = pstg.tile([128, HD], BF16, tag="s11", name="s11", bufs=4)
            nc.scalar.activation(
                s11[:], po[:, off : off + HD], AF.Copy, scale=rc[:, 0:1]
            )
            if qc % 2 == 0:
                trt = psA.tile([128, N], F32, tag="psA", name="psA")
            dst = trt[0:HD, 512 * (qc % 2) : 512 * (qc % 2) + HD].bitcast(BF16)
            nc.tensor.transpose(dst, s11[:], ident[:])
            nc.vector.tensor_copy(ott[KC - 1][HD:128, ts(qc, 128)], dst)
        # final projection step: re-inject partials, add k=5, copy, store.
        # Groups rotate across psP(2) + psO(2) + two psA tiles (4 banks).
        trp = [psA.tile([128, N], F32, tag="psA", name="psA") for _ in range(2)]
        slots = (
            psl
            + [psO.tile([128, 512], F32, tag="psO", name="psF")[:] for _ in range(2)]
            + [t[:, ts(j, 512)] for t in trp for j in range(2)]
        )
        g = 0
        for j in range(NC2):
            for c in range(KC):
                ps = slots[g % 8]
                if g >= 2:
                    nc.tensor.matmul(
                        ps, ident[:], p1t[2 * c + j][:], start=True, stop=False
                    )
                nc.tensor.matmul(
                    ps,
                    wmt[KC - 1][:, ts(c, 128)],
                    ott[KC - 1][:, ts(j, 512)],
                    start=False,
                    stop=True,
                )
                fin = pfin.tile([128, 512], BF16, tag="fin", name="fin")
                if g % 2 == 0:
                    nc.scalar.activation(fin[:], ps, AF.Copy)
                else:
                    nc.vector.tensor_copy(fin[:], ps)
                eng = nc.sync if g % 2 == 0 else nc.scalar
                eng.dma_start(y_d[ts(c, 128), ts(j, 512)], fin[:])
                g += 1


def _build(repeat=1):
    key = ("nc", repeat)
    if key not in _CACHE:
        nc = bacc.Bacc("TRN2", target_bir_lowering=False, debug=False, num_devices=B)
        with tile.TileContext(nc) as tc:
            _emit(tc, repeat=repeat)
        nc.compile()
        _CACHE[key] = nc
    return _CACHE[key]


def kernel(x, weight, W_qkv, W_msa, b_msa):
    nc = _build()
    x = np.asarray(x, dtype=np.float32)
    weight = np.asarray(weight, dtype=np.float32)
    W_qkv = np.asarray(W_qkv, dtype=np.float32)
    bf = ml_dtypes.bfloat16
    # pack [q;k]^T chunk-major: row block m = the six [128,128] stationary
    # pieces of qk chunk m side by side (see _emit)
    wqk = np.ascontiguousarray(
        W_qkv[:, : 2 * D]
        .reshape(KC, 128, 2 * KC, 128)
        .transpose(2, 1, 0, 3)
        .reshape(2 * D, D)
    ).astype(bf)
    wv = np.ascontiguousarray(W_qkv[:, 2 * D :]).astype(bf)
    wm = np.asarray(W_msa, dtype=np.float32).astype(bf)
    ident = np.eye(128, dtype=np.float32).astype(bf)
    in_maps = []
    for b in range(B):
        in_maps.append(
            {
                "xt": np.ascontiguousarray(x[b].T).astype(bf),
                "w": np.ascontiguousarray(weight[b : b + 1]),
                "wqk": wqk,
                "wv": wv,
                "wmsa": wm,
                "bmsa": np.asarray(b_msa, dtype=np.float32),
                "ident": ident,
            }
        )
    res = run_bass_kernel_spmd(nc, in_maps, list(range(B)))
    out = np.stack(
        [res.results[b]["yt"].astype(np.float32).T for b in range(B)], axis=0
    )
    return np.ascontiguousarray(out)
